# revision 31
# baseline (speedup 1.0000x reference)
"""Binarized linear kernel for Trainium2 (8 NeuronCores).

Problem: per-direction binary "match count" GEMM.
  input        (B=64, D=128, I=512)  bool
  weight_noise (D=128, O=512, I=512) bool
  bias_noise   (D=128, O=512)        float32
  out[b,d,o] = (#matches(input[b,d,:], weight_noise[d,:,:]) > bias_noise[d,o])

Math: with +/-1 encoding x~=2x-1, w~=2w-1:
  matches = (I + sum_i x~ w~) / 2, so
  out = (dotpm > 2*bias - I), where dotpm is a single +/-1 GEMM per direction.
Host pre-encodes +/-1 in fp8_e4m3 (exact), precomputes thr = 2*bias - I (exact
in fp32: 2*bias is exact; subtracting 512 from values in [256,1024] is exact by
Sterbenz). PSUM accumulates fp32 exactly (integers <= 512), so the comparison
is bit-identical to the reference.

Sharding: D across the 8 cores (16 directions each), fully independent.

Implementation: raw bacc (no TileContext) — hand-rolled semaphore pipeline
to avoid Tile's ~15us of prologue/epilogue barriers and per-tile semaphore
churn.  Engine roles:
  Sync:   x DMA, 8x w-chunk DMAs (2 directions each), final out DMA
  GpSimd: thr row DMA + partition_broadcast to 64 partitions, sem cleanup
  PE:     per direction: 4 accumulating fp8 matmuls (K=4x128) into a PSUM bank
  DVE:    per direction: psum > thr compare -> uint8
"""

import numpy as np

import sys

for _p in ("/opt/trn_rl_repo",):
    if _p not in sys.path:
        sys.path.insert(0, _p)

B, D, O, I = 64, 128, 512, 512
NCORES = 8
DL = D // NCORES  # directions per core (16)
KC = I // 128     # contraction chunks of 128 (4)
NB = 8            # PSUM banks used (round robin over directions)

_NC_CACHE = {}


def _build_bass():
    import concourse.mybir as mybir
    from concourse import bacc
    import concourse.bass as bass

    fp8 = mybir.dt.float8e4
    f16 = mybir.dt.float16
    f32 = mybir.dt.float32
    u8 = mybir.dt.uint8

    nc = bacc.Bacc("TRN2")
    # DRAM layouts (host pre-arranged, all DMAs fully contiguous):
    #   xt [128, (d c b)]          : xt[k, d, c, b] = xs[b, d0+d, c*128+k]
    #   wt [8, 128, (j c o)]       : wt[p, k, j, c, o] = ws[d0+2p+j, o, c*128+k]
    #   thr [1, (d o)]             : 2*bias - I
    #   out [64, (d o)]
    xt_d = nc.dram_tensor("xt", [128, DL * KC * B], fp8, kind="ExternalInput")
    wt_d = nc.dram_tensor("wt", [DL, 128, KC * O], fp8, kind="ExternalInput")
    thr_d = nc.dram_tensor("thr", [B, DL * O], f16, kind="ExternalInput")
    out_d = nc.dram_tensor("out", [B, DL * O], u8, kind="ExternalOutput")

    from contextlib import ExitStack

    with ExitStack() as ctx:
        x_sb = ctx.enter_context(nc.sbuf_tensor("x_sb", [128, DL * KC * B], fp8))
        w_sb = ctx.enter_context(nc.sbuf_tensor("w_sb", [128, DL * KC * O], fp8))
        thr_sb = ctx.enter_context(nc.sbuf_tensor("thr_sb", [B, DL * O], f16))
        out_sb = ctx.enter_context(nc.sbuf_tensor("out_sb", [B, DL * O], u8))
        warm_x = ctx.enter_context(nc.sbuf_tensor("warm_x", [128, B], fp8))
        warm_w = ctx.enter_context(nc.sbuf_tensor("warm_w", [128, O], fp8))
        psum = ctx.enter_context(nc.psum_tensor([B, NB * O], f32))
        sem_x = [ctx.enter_context(nc.semaphore(f"sem_x{k}")) for k in range(2)]
        sem_w = [ctx.enter_context(nc.semaphore(f"sem_w{k}")) for k in range(DL)]
        sem_thr = ctx.enter_context(nc.semaphore("sem_thr"))
        sem_pe = ctx.enter_context(nc.semaphore("sem_pe"))
        sem_dve = ctx.enter_context(nc.semaphore("sem_dve"))
        sem_out = ctx.enter_context(nc.semaphore("sem_out"))
        block = ctx.enter_context(nc.Block())
        xv = x_sb[:, :].rearrange("k (d c b) -> k d c b", d=DL, c=KC)
        wv = w_sb[:, :].rearrange("k (d c o) -> k d c o", d=DL, c=KC)

        DEPTH = 8  # w-chunk DMAs in flight: all HWDGE queues stay fed
        XH = DL * KC * B // 2

        def wslice(p):
            return w_sb[:, p * KC * O : (p + 1) * KC * O]

        @block.sync
        def _(sync):
            # Even w chunks + x half 0 + output.  Issue is split between the
            # two HWDGE trigger engines (SP here, ACT below) to halve the
            # ~650ns-per-DMA issue ramp.
            sync.dma_start(x_sb[:, 0:XH], xt_d[:, 0:XH]).then_inc(sem_x[0], 16)
            for p in range(0, DL, 2):
                if p >= DEPTH:
                    sync.wait_ge(sem_w[p - DEPTH], 16)
                sync.dma_start(wslice(p), wt_d[p, :, :]).then_inc(sem_w[p], 16)
            # Output in 4 slices so most of the store overlaps compute.
            for q in range(4):
                sync.wait_ge(sem_dve, (q + 1) * DL // 4)
                lo, hi = q * DL * O // 4, (q + 1) * DL * O // 4
                sync.dma_start(out_d[:, lo:hi], out_sb[:, lo:hi]).then_inc(
                    sem_out, 16
                )
            sync.wait_ge(sem_out, 64)

        @block.scalar
        def _(sc):
            # Odd w chunks + x half 1 + thr on the ACT HWDGE trigger.
            sc.dma_start(x_sb[:, XH:], xt_d[:, XH:]).then_inc(sem_x[1], 16)
            for p in range(1, DL, 2):
                if p == 3:
                    sc.dma_start(thr_sb[:, :], thr_d[:, :]).then_inc(sem_thr, 16)
                if p >= DEPTH:
                    sc.wait_ge(sem_w[p - DEPTH], 16)
                sc.dma_start(wslice(p), wt_d[p, :, :]).then_inc(sem_w[p], 16)

        @block.gpsimd
        def _(g):
            # cleanup: after everything retires, reset sems so the NEFF can
            # be re-executed
            g.wait_ge(sem_out, 64)
            all_sems = [*sem_x, *sem_w, sem_thr, sem_pe, sem_dve, sem_out]
            nums = sorted(s.num for s in all_sems)
            lo, hi = nums[0], nums[-1]
            assert nums == list(range(lo, hi + 1)), nums
            g.dma_reset(range(lo, hi + 1))
            g.sem_clear(range(lo, hi + 1))

        N_WARM = 14

        @block.tensor
        def _(t):
            # Warm the PE HAM clock gate on throwaway operands while the
            # first weight chunk streams in (~3.4us of busy time flips the
            # clock from 1.2 to 2.4 GHz).
            for _ in range(N_WARM):
                t.matmul(
                    psum[:, (NB - 1) * O : NB * O],
                    warm_x[:, :],
                    warm_w[:, :],
                    start=True,
                    stop=True,
                )
            t.wait_ge(sem_x[0], 16)
            for d in range(DL):
                if d == DL // 2:
                    t.wait_ge(sem_x[1], 16)
                t.wait_ge(sem_w[d], 16)
                if d >= NB:
                    t.wait_ge(sem_dve, d - NB + 1)
                bank = d % NB
                mm = None
                for c in range(KC):
                    mm = t.matmul(
                        psum[:, bank * O : (bank + 1) * O],
                        xv[:, d, c, :],
                        wv[:, d, c, :],
                        start=(c == 0),
                        stop=(c == KC - 1),
                    )
                mm.then_inc(sem_pe, 1)

        @block.vector
        def _(v):
            v.wait_ge(sem_thr, 16)
            for d in range(DL):
                v.wait_ge(sem_pe, d + 1)
                bank = d % NB
                v.tensor_tensor(
                    out=out_sb[:, d * O : (d + 1) * O],
                    in0=psum[:, bank * O : (bank + 1) * O],
                    in1=thr_sb[:, d * O : (d + 1) * O],
                    op=mybir.AluOpType.is_gt,
                ).then_inc(sem_dve, 1)

    nc.compile()
    return nc


def _get_nc():
    if "nc" not in _NC_CACHE:
        _NC_CACHE["nc"] = _build_bass()
    return _NC_CACHE["nc"]


def _prep_inputs(input, weight_noise, bias_noise):
    import ml_dtypes

    fp8 = ml_dtypes.float8_e4m3
    x = np.asarray(input).astype(np.int8)  # (B, D, I) in {0,1}
    w = np.asarray(weight_noise).astype(np.int8)  # (D, O, I)
    bias = np.asarray(bias_noise).astype(np.float32)  # (D, O)

    xs = (2 * x - 1).astype(fp8)  # +/-1
    ws = (2 * w - 1).astype(fp8)
    # dotpm is always an even integer, so comparing against the odd integer
    # 2*floor(thr/2)+1 gives identical results to comparing against thr —
    # and odd integers |t|<=513 are exact in fp16 (halves thr DMA traffic).
    thr = np.float32(2.0) * bias - np.float32(I)
    thr = (2.0 * np.floor(thr.astype(np.float64) / 2.0) + 1.0).astype(np.float16)

    in_maps = []
    for cidx in range(NCORES):
        dsl = slice(cidx * DL, (cidx + 1) * DL)
        # xt[k, d, c, b] = xs[b, d0+d, c*128+k]
        xt = xs[:, dsl, :].transpose(2, 1, 0)  # (I, DL, B)
        xt = xt.reshape(KC, 128, DL, B).transpose(1, 2, 0, 3)  # (k, d, c, b)
        xt = np.ascontiguousarray(xt).reshape(128, DL * KC * B)
        # wt[d, k, c, o] = ws[d0+d, o, c*128+k]
        wt = ws[dsl].transpose(0, 2, 1)  # (DL, I, O)
        wt = wt.reshape(DL, KC, 128, O).transpose(0, 2, 1, 3)  # (d, k, c, o)
        wt = np.ascontiguousarray(wt).reshape(DL, 128, KC * O)
        th = np.ascontiguousarray(
            np.broadcast_to(thr[dsl].reshape(1, DL * O), (B, DL * O))
        )
        in_maps.append({"xt": xt, "wt": wt, "thr": th})
    return in_maps


def _patch_walrus_args():
    """Cap the semaphore space walrus allocates: its NEFF epilogue clears
    every allocatable semaphore one instruction at a time (~6us for 256)."""
    from concourse import bass_utils as bu

    if getattr(bu, "_max_sem_patched", False):
        return
    orig = bu.get_walrus_args

    def patched(*a, **k):
        return ["--max-sem-num=64", *orig(*a, **k)]

    bu.get_walrus_args = patched
    bu._max_sem_patched = True


def kernel(input, weight_noise, bias_noise):
    from concourse import bass_utils

    _patch_walrus_args()
    in_maps = _prep_inputs(input, weight_noise, bias_noise)
    nc = _get_nc()
    res = bass_utils.run_bass_kernel_spmd(nc, in_maps, core_ids=list(range(NCORES)))
    outs = [np.asarray(r["out"]).reshape(B, DL, O) for r in res.results]
    full = np.concatenate(outs, axis=1)  # (B, D, O)
    return full.astype(bool)


# revision 32
# speedup vs baseline: 1.0640x; 1.0640x over previous
"""Binarized linear kernel for Trainium2 (8 NeuronCores).

Problem: per-direction binary "match count" GEMM.
  input        (B=64, D=128, I=512)  bool
  weight_noise (D=128, O=512, I=512) bool
  bias_noise   (D=128, O=512)        float32
  out[b,d,o] = (#matches(input[b,d,:], weight_noise[d,:,:]) > bias_noise[d,o])

Math: with +/-1 encoding x~=2x-1, w~=2w-1:
  matches = (I + sum_i x~ w~) / 2, so
  out = (dotpm > 2*bias - I), where dotpm is a single +/-1 GEMM per direction.
Host pre-encodes +/-1 in fp8_e4m3 (exact), precomputes thr = 2*bias - I (exact
in fp32: 2*bias is exact; subtracting 512 from values in [256,1024] is exact by
Sterbenz). PSUM accumulates fp32 exactly (integers <= 512), so the comparison
is bit-identical to the reference.

Sharding: D across the 8 cores (16 directions each), fully independent.

Implementation: raw bacc (no TileContext) — hand-rolled semaphore pipeline
to avoid Tile's ~15us of prologue/epilogue barriers and per-tile semaphore
churn.  Engine roles:
  Sync:   x DMA, 8x w-chunk DMAs (2 directions each), final out DMA
  GpSimd: thr row DMA + partition_broadcast to 64 partitions, sem cleanup
  PE:     per direction: 4 accumulating fp8 matmuls (K=4x128) into a PSUM bank
  DVE:    per direction: psum > thr compare -> uint8
"""

import numpy as np

import sys

for _p in ("/opt/trn_rl_repo",):
    if _p not in sys.path:
        sys.path.insert(0, _p)

B, D, O, I = 64, 128, 512, 512
NCORES = 8
DL = D // NCORES  # directions per core (16)
KC = I // 128     # contraction chunks of 128 (4)
NB = 8            # PSUM banks used (round robin over directions)

_NC_CACHE = {}


def _build_bass():
    import concourse.mybir as mybir
    from concourse import bacc
    import concourse.bass as bass

    fp8 = mybir.dt.float8e4
    f16 = mybir.dt.float16
    f32 = mybir.dt.float32
    u8 = mybir.dt.uint8

    nc = bacc.Bacc("TRN2")
    # DRAM layouts (host pre-arranged, all DMAs fully contiguous):
    #   xt [128, (d c b)]          : xt[k, d, c, b] = xs[b, d0+d, c*128+k]
    #   wt [8, 128, (j c o)]       : wt[p, k, j, c, o] = ws[d0+2p+j, o, c*128+k]
    #   thr [1, (d o)]             : 2*bias - I
    #   out [64, (d o)]
    xt_d = nc.dram_tensor("xt", [128, DL * KC * B], fp8, kind="ExternalInput")
    wt_d = nc.dram_tensor("wt", [DL, 128, KC * O], fp8, kind="ExternalInput")
    thr_d = nc.dram_tensor("thr", [B, DL * O], f16, kind="ExternalInput")
    out_d = nc.dram_tensor("out", [B, DL * O], u8, kind="ExternalOutput")

    from contextlib import ExitStack

    with ExitStack() as ctx:
        x_sb = ctx.enter_context(nc.sbuf_tensor("x_sb", [128, DL * KC * B], fp8))
        w_sb = ctx.enter_context(nc.sbuf_tensor("w_sb", [128, DL * KC * O], fp8))
        thr_sb = ctx.enter_context(nc.sbuf_tensor("thr_sb", [B, DL * O], f16))
        out_sb = ctx.enter_context(nc.sbuf_tensor("out_sb", [B, DL * O], u8))
        warm_x = ctx.enter_context(nc.sbuf_tensor("warm_x", [128, B], fp8))
        warm_w = ctx.enter_context(nc.sbuf_tensor("warm_w", [128, O], fp8))
        psum = ctx.enter_context(nc.psum_tensor([B, NB * O], f32))
        sem_x = [ctx.enter_context(nc.semaphore(f"sem_x{k}")) for k in range(2)]
        sem_w = [ctx.enter_context(nc.semaphore(f"sem_w{k}")) for k in range(DL)]
        sem_thr = ctx.enter_context(nc.semaphore("sem_thr"))
        sem_pe = ctx.enter_context(nc.semaphore("sem_pe"))
        sem_dve = ctx.enter_context(nc.semaphore("sem_dve"))
        sem_out = ctx.enter_context(nc.semaphore("sem_out"))
        block = ctx.enter_context(nc.Block())
        xv = x_sb[:, :].rearrange("k (d c b) -> k d c b", d=DL, c=KC)
        wv = w_sb[:, :].rearrange("k (d c o) -> k d c o", d=DL, c=KC)

        DEPTH = 8  # w-chunk DMAs in flight: all HWDGE queues stay fed
        XH = DL * KC * B // 2

        def wslice(p):
            return w_sb[:, p * KC * O : (p + 1) * KC * O]

        @block.sync
        def _(sync):
            # All w chunks on the SP ring: first DEPTH issued immediately,
            # the middle chained (bounds in-flight bytes, keeps completion
            # order tight), and the last DEPTH-1 issued unchained so the tail
            # never becomes issue-gated.
            sync.dma_start(x_sb[:, 0:XH], xt_d[:, 0:XH]).then_inc(sem_x[0], 16)
            for p in range(DL):
                if DEPTH <= p <= DL - DEPTH + 1:
                    sync.wait_ge(sem_w[p - DEPTH], 16)
                sync.dma_start(wslice(p), wt_d[p, :, :]).then_inc(sem_w[p], 16)
            sync.wait_ge(sem_out, 64)

        @block.scalar
        def _(sc):
            # thr + x half 1 + output slices on the ACT HWDGE ring, which is
            # idle during the tail (out never queues behind late w issues).
            sc.dma_start(thr_sb[:, :], thr_d[:, :]).then_inc(sem_thr, 16)
            sc.dma_start(x_sb[:, XH:], xt_d[:, XH:]).then_inc(sem_x[1], 16)
            for q in range(4):
                sc.wait_ge(sem_dve, (q + 1) * DL // 4)
                lo, hi = q * DL * O // 4, (q + 1) * DL * O // 4
                sc.dma_start(out_d[:, lo:hi], out_sb[:, lo:hi]).then_inc(
                    sem_out, 16
                )

        @block.gpsimd
        def _(g):
            # cleanup: after everything retires, reset sems so the NEFF can
            # be re-executed
            g.wait_ge(sem_out, 64)
            all_sems = [*sem_x, *sem_w, sem_thr, sem_pe, sem_dve, sem_out]
            nums = sorted(s.num for s in all_sems)
            lo, hi = nums[0], nums[-1]
            assert nums == list(range(lo, hi + 1)), nums
            g.dma_reset(range(lo, hi + 1))
            g.sem_clear(range(lo, hi + 1))

        N_WARM = 14

        @block.tensor
        def _(t):
            # Warm the PE HAM clock gate on throwaway operands while the
            # first weight chunk streams in (~3.4us of busy time flips the
            # clock from 1.2 to 2.4 GHz).
            for _ in range(N_WARM):
                t.matmul(
                    psum[:, (NB - 1) * O : NB * O],
                    warm_x[:, :],
                    warm_w[:, :],
                    start=True,
                    stop=True,
                )
            t.wait_ge(sem_x[0], 16)
            for d in range(DL):
                if d == DL // 2:
                    t.wait_ge(sem_x[1], 16)
                t.wait_ge(sem_w[d], 16)
                if d >= NB:
                    t.wait_ge(sem_dve, d - NB + 1)
                bank = d % NB
                mm = None
                for c in range(KC):
                    mm = t.matmul(
                        psum[:, bank * O : (bank + 1) * O],
                        xv[:, d, c, :],
                        wv[:, d, c, :],
                        start=(c == 0),
                        stop=(c == KC - 1),
                    )
                mm.then_inc(sem_pe, 1)

        @block.vector
        def _(v):
            v.wait_ge(sem_thr, 16)
            for d in range(DL):
                v.wait_ge(sem_pe, d + 1)
                bank = d % NB
                v.tensor_tensor(
                    out=out_sb[:, d * O : (d + 1) * O],
                    in0=psum[:, bank * O : (bank + 1) * O],
                    in1=thr_sb[:, d * O : (d + 1) * O],
                    op=mybir.AluOpType.is_gt,
                ).then_inc(sem_dve, 1)

    nc.compile()
    return nc


def _get_nc():
    if "nc" not in _NC_CACHE:
        _NC_CACHE["nc"] = _build_bass()
    return _NC_CACHE["nc"]


def _prep_inputs(input, weight_noise, bias_noise):
    import ml_dtypes

    fp8 = ml_dtypes.float8_e4m3
    x = np.asarray(input).astype(np.int8)  # (B, D, I) in {0,1}
    w = np.asarray(weight_noise).astype(np.int8)  # (D, O, I)
    bias = np.asarray(bias_noise).astype(np.float32)  # (D, O)

    xs = (2 * x - 1).astype(fp8)  # +/-1
    ws = (2 * w - 1).astype(fp8)
    # dotpm is always an even integer, so comparing against the odd integer
    # 2*floor(thr/2)+1 gives identical results to comparing against thr —
    # and odd integers |t|<=513 are exact in fp16 (halves thr DMA traffic).
    thr = np.float32(2.0) * bias - np.float32(I)
    thr = (2.0 * np.floor(thr.astype(np.float64) / 2.0) + 1.0).astype(np.float16)

    in_maps = []
    for cidx in range(NCORES):
        dsl = slice(cidx * DL, (cidx + 1) * DL)
        # xt[k, d, c, b] = xs[b, d0+d, c*128+k]
        xt = xs[:, dsl, :].transpose(2, 1, 0)  # (I, DL, B)
        xt = xt.reshape(KC, 128, DL, B).transpose(1, 2, 0, 3)  # (k, d, c, b)
        xt = np.ascontiguousarray(xt).reshape(128, DL * KC * B)
        # wt[d, k, c, o] = ws[d0+d, o, c*128+k]
        wt = ws[dsl].transpose(0, 2, 1)  # (DL, I, O)
        wt = wt.reshape(DL, KC, 128, O).transpose(0, 2, 1, 3)  # (d, k, c, o)
        wt = np.ascontiguousarray(wt).reshape(DL, 128, KC * O)
        th = np.ascontiguousarray(
            np.broadcast_to(thr[dsl].reshape(1, DL * O), (B, DL * O))
        )
        in_maps.append({"xt": xt, "wt": wt, "thr": th})
    return in_maps


def _patch_walrus_args():
    """Cap the semaphore space walrus allocates: its NEFF epilogue clears
    every allocatable semaphore one instruction at a time (~6us for 256)."""
    from concourse import bass_utils as bu

    if getattr(bu, "_max_sem_patched", False):
        return
    orig = bu.get_walrus_args

    def patched(*a, **k):
        return ["--max-sem-num=64", *orig(*a, **k)]

    bu.get_walrus_args = patched
    bu._max_sem_patched = True


def kernel(input, weight_noise, bias_noise):
    from concourse import bass_utils

    _patch_walrus_args()
    in_maps = _prep_inputs(input, weight_noise, bias_noise)
    nc = _get_nc()
    res = bass_utils.run_bass_kernel_spmd(nc, in_maps, core_ids=list(range(NCORES)))
    outs = [np.asarray(r["out"]).reshape(B, DL, O) for r in res.results]
    full = np.concatenate(outs, axis=1)  # (B, D, O)
    return full.astype(bool)


# revision 33
# speedup vs baseline: 1.3393x; 1.2587x over previous
"""Binarized linear kernel for Trainium2 (8 NeuronCores).

Problem: per-direction binary "match count" GEMM.
  input        (B=64, D=128, I=512)  bool
  weight_noise (D=128, O=512, I=512) bool
  bias_noise   (D=128, O=512)        float32
  out[b,d,o] = (#matches(input[b,d,:], weight_noise[d,:,:]) > bias_noise[d,o])

Math: with +/-1 encoding x~=2x-1, w~=2w-1:
  matches = (I + sum_i x~ w~) / 2, so
  out = (dotpm > 2*bias - I), where dotpm is a single +/-1 GEMM per direction.
Host pre-encodes +/-1 in fp8_e4m3 (exact).  dotpm is always an even integer,
so comparing against the odd integer 2*floor(thr/2)+1 is exactly equivalent
and fits fp16 losslessly (|t| <= 513 < 2048).  PSUM accumulates fp32 exactly
(integers <= 512), so results are bit-identical to the reference.

Sharding: D across the 8 cores (16 directions each), fully independent.

Implementation: raw bacc (no TileContext), hand-rolled semaphore pipeline.
Directions are processed in PAIRS packed into the two 64-column halves of the
128x128 PE array via tile_position, so both matmuls of a pair run
concurrently (2x PE throughput at M=64) and the epilogue compare uses all
128 DVE lanes.  Engine roles:
  SP:  x half DMA + 8 w pair-chunk DMAs (512KB each, middle ones chained)
  ACT: thr DMA, x half DMA, 4 output-slice DMAs
  PE:  HAM warm-up matmuls, then per pair: 2x4 accumulating fp8 matmuls
  DVE: per pair: psum[128,512] > thr -> uint8
  POOL: final semaphore cleanup
"""

import numpy as np

import sys

for _p in ("/opt/trn_rl_repo",):
    if _p not in sys.path:
        sys.path.insert(0, _p)

B, D, O, I = 64, 128, 512, 512
NCORES = 8
DL = D // NCORES   # directions per core (16)
NP = DL // 2       # direction pairs per core (8)
KC = I // 128      # contraction chunks of 128 (4)
NB = 4             # PSUM banks of [128, 512] used round robin

_NC_CACHE = {}


def _build_bass():
    import concourse.mybir as mybir
    from concourse import bacc

    fp8 = mybir.dt.float8e4
    f16 = mybir.dt.float16
    u8 = mybir.dt.uint8
    f32 = mybir.dt.float32

    nc = bacc.Bacc("TRN2")
    # DRAM layouts (host pre-arranged, DMAs fully contiguous):
    #   xt  [128, (p c j b)] : xt[k, p, c, j, b] = xs[b, d0+2p+j, c*128+k]
    #   wt  [NP, 128, (j c o)] : wt[p, k, j, c, o] = ws[d0+2p+j, o, c*128+k]
    #   thr [128, (p o)] f16 : rows 0:64 even dir of pair, 64:128 odd dir
    #   out [128, (p o)] u8  : same row split; host unscrambles
    xt_d = nc.dram_tensor("xt", [128, NP * KC * 2 * B], fp8, kind="ExternalInput")
    wt_d = nc.dram_tensor("wt", [NP, 128, 2 * KC * O], fp8, kind="ExternalInput")
    thr_d = nc.dram_tensor("thr", [128, NP * O], f16, kind="ExternalInput")
    out_d = nc.dram_tensor("out", [128, NP * O], u8, kind="ExternalOutput")

    from contextlib import ExitStack

    with ExitStack() as ctx:
        x_sb = ctx.enter_context(nc.sbuf_tensor("x_sb", [128, NP * KC * 2 * B], fp8))
        w_sb = ctx.enter_context(nc.sbuf_tensor("w_sb", [128, NP * 2 * KC * O], fp8))
        thr_sb = ctx.enter_context(nc.sbuf_tensor("thr_sb", [128, NP * O], f16))
        out_sb = ctx.enter_context(nc.sbuf_tensor("out_sb", [128, NP * O], u8))
        warm_x = ctx.enter_context(nc.sbuf_tensor("warm_x", [128, B], fp8))
        warm_w = ctx.enter_context(nc.sbuf_tensor("warm_w", [128, O], fp8))
        psum = ctx.enter_context(nc.psum_tensor([128, NB * O], f32))
        sem_x = [ctx.enter_context(nc.semaphore(f"sem_x{k}")) for k in range(2)]
        sem_w = [ctx.enter_context(nc.semaphore(f"sem_w{k}")) for k in range(NP)]
        sem_thr = ctx.enter_context(nc.semaphore("sem_thr"))
        sem_pe = ctx.enter_context(nc.semaphore("sem_pe"))
        sem_dve = ctx.enter_context(nc.semaphore("sem_dve"))
        sem_out = ctx.enter_context(nc.semaphore("sem_out"))
        block = ctx.enter_context(nc.Block())

        xv = x_sb[:, :].rearrange("k (p c j b) -> k p c j b", p=NP, c=KC, j=2)
        wv = w_sb[:, :].rearrange("k (p j c o) -> k p j c o", p=NP, j=2, c=KC)

        DEPTH = 4  # w pair-chunk DMAs in flight
        XH = NP * KC * 2 * B // 2
        CW = 2 * KC * O  # free bytes per pair chunk

        @block.sync
        def _(sync):
            # w chunks: first DEPTH immediate, middle chained (bounds
            # in-flight bytes -> tight completion order), tail unchained so
            # it never goes issue-gated.
            sync.dma_start(x_sb[:, 0:XH], xt_d[:, 0:XH]).then_inc(sem_x[0], 16)
            for p in range(NP):
                if DEPTH <= p <= NP - DEPTH + 1:
                    sync.wait_ge(sem_w[p - DEPTH], 16)
                sync.dma_start(
                    w_sb[:, p * CW : (p + 1) * CW], wt_d[p, :, :]
                ).then_inc(sem_w[p], 16)
            sync.wait_ge(sem_out, 64)

        @block.scalar
        def _(sc):
            # thr + x half 1 + output slices on the ACT HWDGE ring (idle at
            # the tail, so out never queues behind late w issues).
            sc.dma_start(thr_sb[:, :], thr_d[:, :]).then_inc(sem_thr, 16)
            sc.dma_start(x_sb[:, XH:], xt_d[:, XH:]).then_inc(sem_x[1], 16)
            for q in range(4):
                sc.wait_ge(sem_dve, (q + 1) * NP // 4)
                lo, hi = q * NP * O // 4, (q + 1) * NP * O // 4
                sc.dma_start(out_d[:, lo:hi], out_sb[:, lo:hi]).then_inc(
                    sem_out, 16
                )

        @block.gpsimd
        def _(g):
            # cleanup: reset sems so the NEFF can be re-executed
            g.wait_ge(sem_out, 64)
            all_sems = [*sem_x, *sem_w, sem_thr, sem_pe, sem_dve, sem_out]
            nums = sorted(s.num for s in all_sems)
            lo, hi = nums[0], nums[-1]
            assert nums == list(range(lo, hi + 1)), nums
            g.dma_reset(range(lo, hi + 1))
            g.sem_clear(range(lo, hi + 1))

        N_WARM = 12

        @block.tensor
        def _(t):
            # Warm the PE HAM clock gate on throwaway operands while the
            # first chunks stream in (~3.4us busy flips 1.2 -> 2.4 GHz).
            for _ in range(N_WARM):
                t.matmul(
                    psum[0:B, (NB - 1) * O : NB * O],
                    warm_x[:, :],
                    warm_w[:, :],
                    start=True,
                    stop=True,
                )
            t.wait_ge(sem_x[0], 16)
            for p in range(NP):
                if p == NP // 2:
                    t.wait_ge(sem_x[1], 16)
                t.wait_ge(sem_w[p], 16)
                if p >= NB:
                    t.wait_ge(sem_dve, p - NB + 1)
                bank = p % NB
                bank_ap = psum[:, bank * O : (bank + 1) * O]
                mm = None
                for c in range(KC):
                    # even direction -> array columns 0:64, psum rows 0:64
                    t.matmul(
                        bank_ap[0:B, :],
                        xv[:, p, c, 0, :],
                        wv[:, p, 0, c, :],
                        start=(c == 0),
                        stop=(c == KC - 1),
                        tile_position=(0, 0),
                    )
                    # odd direction -> array columns 64:128, psum rows 64:128
                    mm = t.matmul(
                        bank_ap[B : 2 * B, :],
                        xv[:, p, c, 1, :],
                        wv[:, p, 1, c, :],
                        start=(c == 0),
                        stop=(c == KC - 1),
                        tile_position=(0, 64),
                    )
                mm.then_inc(sem_pe, 1)

        @block.vector
        def _(v):
            v.wait_ge(sem_thr, 16)
            for p in range(NP):
                v.wait_ge(sem_pe, p + 1)
                bank = p % NB
                v.tensor_tensor(
                    out=out_sb[:, p * O : (p + 1) * O],
                    in0=psum[:, bank * O : (bank + 1) * O],
                    in1=thr_sb[:, p * O : (p + 1) * O],
                    op=mybir.AluOpType.is_gt,
                ).then_inc(sem_dve, 1)

    nc.compile()
    return nc


def _get_nc():
    if "nc" not in _NC_CACHE:
        _NC_CACHE["nc"] = _build_bass()
    return _NC_CACHE["nc"]


def _prep_inputs(input, weight_noise, bias_noise):
    import ml_dtypes

    fp8 = ml_dtypes.float8_e4m3
    x = np.asarray(input).astype(np.int8)  # (B, D, I) in {0,1}
    w = np.asarray(weight_noise).astype(np.int8)  # (D, O, I)
    bias = np.asarray(bias_noise).astype(np.float32)  # (D, O)

    xs = (2 * x - 1).astype(fp8)  # +/-1
    ws = (2 * w - 1).astype(fp8)
    # dotpm is even; the odd integer 2*floor(thr/2)+1 compares identically
    # and is exact in fp16.
    thr = np.float32(2.0) * bias - np.float32(I)
    thr = (2.0 * np.floor(thr.astype(np.float64) / 2.0) + 1.0).astype(np.float16)

    in_maps = []
    for cidx in range(NCORES):
        dsl = slice(cidx * DL, (cidx + 1) * DL)
        # xt[k, p, c, j, b] = xs[b, d0+2p+j, c*128+k]
        xt = xs[:, dsl, :].transpose(2, 1, 0)  # (I, DL, B)
        xt = xt.reshape(KC, 128, NP, 2, B)  # (c, k, p, j, b)
        xt = xt.transpose(1, 2, 0, 3, 4)  # (k, p, c, j, b)
        xt = np.ascontiguousarray(xt).reshape(128, NP * KC * 2 * B)
        # wt[p, k, j, c, o] = ws[d0+2p+j, o, c*128+k]
        wt = ws[dsl].transpose(0, 2, 1)  # (DL, I, O)
        wt = wt.reshape(NP, 2, KC, 128, O)  # (p, j, c, k, o)
        wt = wt.transpose(0, 3, 1, 2, 4)  # (p, k, j, c, o)
        wt = np.ascontiguousarray(wt).reshape(NP, 128, 2 * KC * O)
        # thr rows 0:64 = even dir of pair, 64:128 = odd dir
        th = thr[dsl].reshape(NP, 2, O)
        th_lo = np.broadcast_to(th[:, 0, :].reshape(1, NP, O), (B, NP, O))
        th_hi = np.broadcast_to(th[:, 1, :].reshape(1, NP, O), (B, NP, O))
        thp = np.concatenate([th_lo, th_hi], axis=0)  # (128, NP, O)
        thp = np.ascontiguousarray(thp).reshape(128, NP * O)
        in_maps.append({"xt": xt, "wt": wt, "thr": thp})
    return in_maps


def _patch_walrus_args():
    from concourse import bass_utils as bu

    if getattr(bu, "_max_sem_patched", False):
        return
    orig = bu.get_walrus_args

    def patched(*a, **k):
        return ["--max-sem-num=64", *orig(*a, **k)]

    bu.get_walrus_args = patched
    bu._max_sem_patched = True


def kernel(input, weight_noise, bias_noise):
    from concourse import bass_utils

    _patch_walrus_args()
    in_maps = _prep_inputs(input, weight_noise, bias_noise)
    nc = _get_nc()
    res = bass_utils.run_bass_kernel_spmd(nc, in_maps, core_ids=list(range(NCORES)))
    full = np.empty((B, D, O), dtype=bool)
    for cidx, r in enumerate(res.results):
        ro = np.asarray(r["out"]).reshape(128, NP, O)
        dsl = slice(cidx * DL, (cidx + 1) * DL)
        blk = full[:, dsl, :]
        blk[:, 0::2, :] = ro[0:B].astype(bool)
        blk[:, 1::2, :] = ro[B : 2 * B].astype(bool)
    return full
